# revision 1
# baseline (speedup 1.0000x reference)
"""Multi-head attention (B=2, T=4096, D=512, H=8) on 8 TRN2 NeuronCores.

Sharding: core c handles batch c//4 and query rows (c%4)*1024..+1024.
Heads stay together on a core; K/V are recomputed per core (no comm).

Per-core device kernel (all matmuls bf16 with fp32 PSUM accumulation):
  phase A: project K^T (feature-major), V (token-major, with a ones
           column appended per head for the softmax denominator), and
           Q^T for the core's own query rows.
  phase B: per 512-query block / head pair / 128-key chunk:
           S^T = K^T.T @ Q^T (keys on partitions, queries on free dim;
           the two heads of a pair run as concurrent 64-row PE tiles),
           P = exp(0.125 * S^T) on ScalarE, P *= mask01 on VectorE
           (masked weights become exactly 0, like the reference where
           exp(-1e9) underflows), O^T(+denom row) += V_aug.T @ P.
           Normalize by the denominator row (fast reciprocal + ones-row
           matmul broadcast).
           After all pairs of a query block: out = O @ Wo for that
           block, overlapping the next block's attention.
"""

import sys

sys.path.insert(0, "/opt/trn_rl_repo")

import numpy as np
import ml_dtypes

B, T, D, H = 2, 4096, 512, 8
DH = D // H          # 64
N_CORES = 8
QPC = 1024           # query rows per core
DC = D // 128        # 4 partition chunks of the model dim
KC = T // 128        # 32 key chunks

_BUILT = {}


def _build(with_bias: bool):
    from concourse import bacc
    import concourse.mybir as mybir
    import concourse.tile as tile

    dt = mybir.dt
    AF = mybir.ActivationFunctionType

    nc = bacc.Bacc("TRN2", target_bir_lowering=False, debug=False,
                   num_devices=N_CORES)

    xT = nc.dram_tensor("xT", [128, DC, T], dt.bfloat16, kind="ExternalInput").ap()
    xQT = nc.dram_tensor("xQT", [128, DC, QPC], dt.bfloat16, kind="ExternalInput").ap()
    wq = nc.dram_tensor("wq", [128, DC, D], dt.bfloat16, kind="ExternalInput").ap()
    wk = nc.dram_tensor("wk", [128, DC, D], dt.bfloat16, kind="ExternalInput").ap()
    wv = nc.dram_tensor("wv", [128, DC, D], dt.bfloat16, kind="ExternalInput").ap()
    wo = nc.dram_tensor("wo", [128, DC, D], dt.bfloat16, kind="ExternalInput").ap()
    mska = nc.dram_tensor("mska", [128, KC // 2, QPC], dt.bfloat16, kind="ExternalInput").ap()
    mskb = nc.dram_tensor("mskb", [128, KC // 2, QPC], dt.bfloat16, kind="ExternalInput").ap()
    sel2 = nc.dram_tensor("sel2", [2, 128], dt.float16, kind="ExternalInput").ap()
    if with_bias:
        bqkd = nc.dram_tensor("bqk", [128, DC, 2], dt.float32, kind="ExternalInput").ap()
        bvo = nc.dram_tensor("bvo", [1, 2, D], dt.bfloat16, kind="ExternalInput").ap()
        ones1 = nc.dram_tensor("ones1", [1, 128], dt.bfloat16, kind="ExternalInput").ap()
    out = nc.dram_tensor("out", [QPC, D], dt.float32, kind="ExternalOutput").ap()

    with tile.TileContext(nc) as tc:
        with tc.tile_pool(name="persist", bufs=1) as pp:
            wq_sb = pp.tile([128, DC, D], dt.bfloat16, tag="wq")
            wk_sb = pp.tile([128, DC, D], dt.bfloat16, tag="wk")
            wv_sb = pp.tile([128, DC, D], dt.bfloat16, tag="wv")
            wo_sb = pp.tile([128, DC, D], dt.bfloat16, tag="wo")
            mska_sb = pp.tile([128, KC // 2, QPC], dt.bfloat16, tag="mska")
            sel2_sb = pp.tile([2, 128], dt.float16, tag="sel2")
            kt_f = [pp.tile([128, T], dt.bfloat16, tag=f"kt{fo}", name=f"kt{fo}")
                    for fo in range(DC)]
            v_g = [pp.tile([128, KC // 4, H, DH + 1], dt.bfloat16, tag=f"v{g}", name=f"v{g}")
                   for g in range(4)]
            qt_f = [pp.tile([128, QPC], dt.bfloat16, tag=f"qt{fo}", name=f"qt{fo}")
                    for fo in range(DC)]
            ot_f = [pp.tile([128, QPC], dt.bfloat16, tag=f"ot{fo}", name=f"ot{fo}")
                    for fo in range(DC)]

            # ---------------- phase A: projections ----------------
            with (
                tc.tile_pool(name="xp", bufs=1) as xp,
                tc.tile_pool(name="psA", bufs=4, space="PSUM") as psA,
            ):
                xt_sb = xp.tile([128, DC, T], dt.bfloat16, tag="xt")
                xqt_sb = xp.tile([128, DC, QPC], dt.bfloat16, tag="xqt")
                # DMA issue order = arrival order: activations first so
                # the projection matmuls start ASAP, mask halves last.
                nc.sync.dma_start(xt_sb[:], xT[:])
                nc.sync.dma_start(xqt_sb[:], xQT[:])
                for sb, dr in ((wk_sb, wk), (wq_sb, wq), (wv_sb, wv),
                               (wo_sb, wo), (sel2_sb, sel2)):
                    nc.sync.dma_start(sb[:], dr[:])
                if with_bias:
                    bqk_sb = pp.tile([128, DC, 2], dt.float32, tag="bqk")
                    bvo_sb = pp.tile([1, 2, D], dt.bfloat16, tag="bvo")
                    ones1_sb = pp.tile([1, 128], dt.bfloat16, tag="ones1")
                    nc.sync.dma_start(bqk_sb[:], bqkd[:])
                    nc.sync.dma_start(bvo_sb[:], bvo[:])
                    nc.sync.dma_start(ones1_sb[:], ones1[:])
                nc.sync.dma_start(mska_sb[:], mska[:])
                # ones column of V_aug (denominator accumulator)
                for g in range(4):
                    nc.vector.memset(v_g[g][:, :, :, DH:DH + 1], 1.0)

                def proj_tile(w_sb, src_sb, out_ap, nb, bi, fo):
                    ps = psA.tile([128, 512], dt.float32, tag="psA")
                    for dc in range(DC):
                        nc.tensor.matmul(
                            ps[:],
                            w_sb[:, dc, fo * 128:(fo + 1) * 128],
                            src_sb[:, dc, nb * 512:(nb + 1) * 512],
                            start=(dc == 0), stop=(dc == DC - 1),
                        )
                    if with_bias:
                        nc.vector.tensor_scalar(
                            out_ap, ps[:], bqk_sb[:, fo, bi:bi + 1], 0.0,
                            mybir.AluOpType.add, mybir.AluOpType.bypass,
                        )
                    else:
                        nc.scalar.copy(out_ap, ps[:])

                def v_tile(tt):
                    ps = psA.tile([128, 512], dt.float32, tag="psA")
                    for dc in range(DC):
                        nc.tensor.matmul(
                            ps[:],
                            xt_sb[:, dc, tt * 128:(tt + 1) * 128],
                            wv_sb[:, dc, :],
                            start=(dc == 0),
                            stop=(not with_bias and dc == DC - 1),
                        )
                    if with_bias:
                        nc.tensor.matmul(ps[:], ones1_sb[:], bvo_sb[:, 0, :],
                                         start=False, stop=True)
                    nc.vector.tensor_copy(
                        v_g[tt // 8][:, tt % 8, :, 0:DH],
                        ps[:].rearrange("p (h f) -> p h f", h=H))

                # emit in the order phase B consumes: first the K/Q
                # chunks of head pair 0, then all of V (so the O^T
                # accumulation never head-of-line-blocks the PE), then
                # the remaining K/Q chunks.
                def kq_group(fo):
                    for nb in range(T // 512):
                        proj_tile(wk_sb, xt_sb,
                                  kt_f[fo][:, nb * 512:(nb + 1) * 512], nb, 1, fo)
                    for nb in range(QPC // 512):
                        proj_tile(wq_sb, xqt_sb,
                                  qt_f[fo][:, nb * 512:(nb + 1) * 512], nb, 0, fo)

                kq_group(0)
                for tt in range(KC):
                    v_tile(tt)
                for fo_ in range(1, DC):
                    kq_group(fo_)

            # ------------- phase B: attention + output projection -------------
            with (
                tc.tile_pool(name="m2", bufs=1) as m2,
                tc.tile_pool(name="psBC", bufs=1, space="PSUM") as psBC,
                tc.tile_pool(name="psC", bufs=1, space="PSUM") as psC,
                tc.tile_pool(name="psS", bufs=2, space="PSUM") as psS,
                tc.tile_pool(name="psO", bufs=1, space="PSUM") as psO,
                tc.tile_pool(name="pB", bufs=4) as pB,
                tc.tile_pool(name="pN", bufs=3) as pN,
                tc.tile_pool(name="pC", bufs=2) as pC,
            ):
                mskb_sb = m2.tile([128, KC // 2, QPC], dt.bfloat16, tag="mskb")
                nc.sync.dma_start(mskb_sb[:], mskb[:])
                msk_half = (mska_sb, mskb_sb)
                for jq in range(QPC // 512):
                    qs = slice(jq * 512, (jq + 1) * 512)
                    for pr in range(H // 2):
                        ot_e = psO.tile([DH + 1, 512], dt.float32, tag="ote")
                        ot_o = psO.tile([DH + 1, 512], dt.float32, tag="oto")
                        for kc in range(KC):
                            ks = slice(kc * 128, (kc + 1) * 128)
                            sp = psS.tile([128, 1024], dt.float32, tag="sp")
                            # scores^T for the even/odd head of the pair
                            nc.tensor.matmul(
                                sp[:, 0:512],
                                kt_f[pr][0:64, ks], qt_f[pr][0:64, qs],
                                start=True, stop=True)
                            nc.tensor.matmul(
                                sp[:, 512:1024],
                                kt_f[pr][64:128, ks], qt_f[pr][64:128, qs],
                                start=True, stop=True)
                            p_sb = pB.tile([128, 1024], dt.bfloat16, tag="p")
                            nc.scalar.activation(p_sb[:], sp[:], AF.Exp,
                                                 scale=0.125)
                            # zero the masked weights (mask tile shared by pair)
                            mh = msk_half[kc // (KC // 2)]
                            pv = p_sb[:].rearrange("p (a b) -> p a b", a=2)
                            nc.vector.tensor_mul(
                                pv, pv,
                                mh[:, kc % (KC // 2), qs][:, None, :]
                                .to_broadcast((128, 2, 512)))
                            nc.tensor.matmul(ot_e[:], v_g[kc // 8][:, kc % 8, 2 * pr, :],
                                             p_sb[:, 0:512],
                                             start=(kc == 0), stop=(kc == KC - 1))
                            nc.tensor.matmul(ot_o[:], v_g[kc // 8][:, kc % 8, 2 * pr + 1, :],
                                             p_sb[:, 512:1024],
                                             start=(kc == 0), stop=(kc == KC - 1))
                        # normalize both heads by their denominator row.
                        # Copy numerator+denominator out of PSUM right away
                        # (frees the OT banks for the next pair), then fast
                        # recip + ones-row matmul broadcast + multiply.
                        for i, ot_ps in ((0, ot_e), (1, ot_o)):
                            den = pN.tile([1, 512], dt.float32, tag="den")
                            nc.scalar.copy(den[:], ot_ps[DH:DH + 1, :])
                            rec = pN.tile([1, 512], dt.float32, tag="rec")
                            nc.vector.reciprocal_approx_fast(rec[:], den[:])
                            rech = pN.tile([1, 512], dt.float16, tag="rech")
                            nc.vector.tensor_copy(rech[:], rec[:])
                            bc = psBC.tile([64, 512], dt.float32, tag="bc")
                            nc.tensor.matmul(bc[:], sel2_sb[0:1, 0:64],
                                             rech[:], start=True, stop=True)
                            bcs = pN.tile([64, 512], dt.float32, tag="bcs")
                            nc.scalar.copy(bcs[:], bc[:])
                            nc.vector.tensor_mul(
                                ot_f[pr][i * 64:(i + 1) * 64, qs],
                                ot_ps[0:DH, :], bcs[:])

                    # output projection for this query block (overlaps the
                    # next block's attention)
                    for tt in range(jq * 4, (jq + 1) * 4):
                        ps = psC.tile([128, 512], dt.float32, tag="psC")
                        for dc in range(DC):
                            nc.tensor.matmul(
                                ps[:],
                                ot_f[dc][:, tt * 128:(tt + 1) * 128],
                                wo_sb[:, dc, :],
                                start=(dc == 0),
                                stop=(with_bias is False and dc == DC - 1))
                        if with_bias:
                            nc.tensor.matmul(ps[:], ones1_sb[:],
                                             bvo_sb[:, 1, :],
                                             start=False, stop=True)
                        os = pC.tile([128, 512], dt.float32, tag="os")
                        nc.vector.tensor_copy(os[:], ps[:])
                        nc.sync.dma_start(out[tt * 128:(tt + 1) * 128, :], os[:])

    nc.compile()
    return nc


def _get_nc(with_bias: bool):
    if with_bias not in _BUILT:
        _BUILT[with_bias] = _build(with_bias)
    return _BUILT[with_bias]


def _prep_inputs(x, Wq, bq, Wk, bk, Wv, bv, Wo, bo, mask, with_bias):
    bf16 = ml_dtypes.bfloat16

    shared = {}
    for name, W in (("wq", Wq), ("wk", Wk), ("wv", Wv), ("wo", Wo)):
        shared[name] = np.ascontiguousarray(
            np.asarray(W, np.float32).astype(bf16)
            .reshape(DC, 128, D).transpose(1, 0, 2))
    shared["sel2"] = np.ones((2, 128), np.float32).astype(np.float16)
    if with_bias:
        shared["bqk"] = np.ascontiguousarray(np.stack(
            [np.asarray(bq, np.float32).reshape(DC, 128).T,
             np.asarray(bk, np.float32).reshape(DC, 128).T], axis=-1))
        shared["bvo"] = np.ascontiguousarray(np.stack(
            [np.asarray(bv, np.float32), np.asarray(bo, np.float32)]
        ).astype(bf16).reshape(1, 2, D))
        shared["ones1"] = np.ones((1, 128), np.float32).astype(bf16)

    maskT = np.asarray(mask).reshape(T, T).T          # (k, q)
    m01T = maskT.astype(np.float32)

    in_maps = []
    for c in range(N_CORES):
        b, qlo = c // 4, (c % 4) * QPC
        xTb = np.asarray(x[b], np.float32).T.astype(bf16)     # (D, T)
        m = dict(shared)
        m["xT"] = np.ascontiguousarray(
            xTb.reshape(DC, 128, T).transpose(1, 0, 2))
        m["xQT"] = np.ascontiguousarray(
            xTb[:, qlo:qlo + QPC].reshape(DC, 128, QPC).transpose(1, 0, 2))
        m01 = np.ascontiguousarray(
            m01T[:, qlo:qlo + QPC].reshape(KC, 128, QPC)
            .transpose(1, 0, 2)).astype(bf16)
        m["mska"] = np.ascontiguousarray(m01[:, :KC // 2])
        m["mskb"] = np.ascontiguousarray(m01[:, KC // 2:])
        in_maps.append(m)
    return in_maps


def _run(inputs, trace=False):
    from concourse.bass_utils import run_bass_kernel_spmd

    with_bias = any(
        float(np.abs(np.asarray(inputs[k], np.float32)).max()) != 0.0
        for k in ("bq", "bk", "bv", "bo"))
    nc = _get_nc(with_bias)
    in_maps = _prep_inputs(
        inputs["x"], inputs["Wq"], inputs["bq"], inputs["Wk"], inputs["bk"],
        inputs["Wv"], inputs["bv"], inputs["Wo"], inputs["bo"],
        inputs["mask"], with_bias)
    res = run_bass_kernel_spmd(nc, in_maps, list(range(N_CORES)), trace=trace)
    O = np.empty((B, T, D), np.float32)
    for c in range(N_CORES):
        b, qlo = c // 4, (c % 4) * QPC
        O[b, qlo:qlo + QPC, :] = res.results[c]["out"]
    return O, res


def kernel(**inputs) -> np.ndarray:
    out, _ = _run(inputs, trace=False)
    return out



# revision 8
# speedup vs baseline: 1.0232x; 1.0232x over previous
"""Multi-head attention (B=2, T=4096, D=512, H=8) on 8 TRN2 NeuronCores.

Sharding: core c handles batch c//4 and query rows (c%4)*1024..+1024.
Heads stay together on a core; K/V are recomputed per core (no comm).

v2 layout: single fused stream — the K/Q/V projection tiles are emitted
just-in-time inside the attention round loop so the ScalarE exp stream
(the critical engine, ~1 elem/lane/cycle over 33.5M elems/core) starts
a few microseconds in instead of after the whole projection phase.
ScalarE runs exp only; every PSUM->SBUF copy and the softmax
normalization run on VectorE (reciprocal_approx_fast reads the
denominator row straight from PSUM; one 2-row selector matmul
broadcasts both heads' reciprocals across partitions).

Per-round dataflow (jq query half, pr head pair, kc key chunk):
  S^T pair = kt[0:64].T@qt[0:64] | kt[64:128].T@qt[64:128]  (row-tiled,
  concurrent 64-row PE tiles) -> exp(0.125 x) on ScalarE -> mask01
  multiply on VectorE (bf16, 2x mode) -> O^T(+denominator row) +=
  V_aug.T @ P accumulated over all 32 kc in PSUM.
"""

import sys

sys.path.insert(0, "/opt/trn_rl_repo")

import numpy as np
import ml_dtypes

B, T, D, H = 2, 4096, 512, 8
DH = D // H          # 64
N_CORES = 8
QPC = 1024           # query rows per core
DC = D // 128        # 4 partition chunks of the model dim
KC = T // 128        # 32 key chunks
NMB = 9              # rotating mask buffers of [128, 4kc, 512q]

_BUILT = {}


def _build(with_bias: bool):
    from concourse import bacc
    import concourse.mybir as mybir
    import concourse.tile as tile

    dt = mybir.dt
    AF = mybir.ActivationFunctionType

    nc = bacc.Bacc("TRN2", target_bir_lowering=False, debug=False,
                   num_devices=N_CORES)

    xT = nc.dram_tensor("xT", [128, DC, T], dt.bfloat16, kind="ExternalInput").ap()
    xQT = nc.dram_tensor("xQT", [128, DC, QPC], dt.bfloat16, kind="ExternalInput").ap()
    wq = nc.dram_tensor("wq", [128, DC, D], dt.bfloat16, kind="ExternalInput").ap()
    wk = nc.dram_tensor("wk", [128, DC, D], dt.bfloat16, kind="ExternalInput").ap()
    wv = nc.dram_tensor("wv", [128, DC, D], dt.bfloat16, kind="ExternalInput").ap()
    wo = nc.dram_tensor("wo", [128, DC, D], dt.bfloat16, kind="ExternalInput").ap()
    # mask chunks: index (jq*8+g) -> [128, 4 kc, 512 q] as 0/1 bf16
    msk = nc.dram_tensor("msk", [16, 128, 4, 512], dt.bfloat16, kind="ExternalInput").ap()
    sel = nc.dram_tensor("sel", [2, 128], dt.float16, kind="ExternalInput").ap()
    if with_bias:
        bqkd = nc.dram_tensor("bqk", [128, DC, 2], dt.float32, kind="ExternalInput").ap()
        bvo = nc.dram_tensor("bvo", [1, 2, D], dt.bfloat16, kind="ExternalInput").ap()
        ones1 = nc.dram_tensor("ones1", [1, 128], dt.bfloat16, kind="ExternalInput").ap()
    out = nc.dram_tensor("out", [QPC, D], dt.float32, kind="ExternalOutput").ap()

    with tile.TileContext(nc) as tc:
        with (
            tc.tile_pool(name="persist", bufs=1) as pp,
            tc.tile_pool(name="psS", bufs=2, space="PSUM") as psS,
            tc.tile_pool(name="psO", bufs=1, space="PSUM") as psO,
            tc.tile_pool(name="psX", bufs=2, space="PSUM") as psX,
            tc.tile_pool(name="pP", bufs=3) as pP,
            tc.tile_pool(name="pN", bufs=2) as pN,
            tc.tile_pool(name="pC", bufs=2) as pC,
        ):
            wq_sb = pp.tile([128, DC, D], dt.bfloat16, tag="wq")
            wk_sb = pp.tile([128, DC, D], dt.bfloat16, tag="wk")
            wv_sb = pp.tile([128, DC, D], dt.bfloat16, tag="wv")
            wo_sb = pp.tile([128, DC, D], dt.bfloat16, tag="wo")
            sel_sb = pp.tile([2, 128], dt.float16, tag="sel")
            xt_sb = pp.tile([128, DC, T], dt.bfloat16, tag="xt")
            xqt_sb = pp.tile([128, DC, QPC], dt.bfloat16, tag="xqt")
            kt_f = [pp.tile([128, T], dt.bfloat16, tag=f"kt{fo}", name=f"kt{fo}")
                    for fo in range(DC)]
            v_g = [pp.tile([128, KC // 4, H, DH + 1], dt.bfloat16, tag=f"v{g}", name=f"v{g}")
                   for g in range(4)]
            qt_f = [pp.tile([128, QPC], dt.bfloat16, tag=f"qt{fo}", name=f"qt{fo}")
                    for fo in range(DC)]
            ot_f = [pp.tile([128, QPC], dt.bfloat16, tag=f"ot{fo}", name=f"ot{fo}")
                    for fo in range(DC)]
            mt = [pp.tile([128, 4, 512], dt.bfloat16, tag=f"m{i}", name=f"m{i}")
                  for i in range(NMB)]

            # ---- DMA issue order = arrival order ----
            nc.sync.dma_start(wk_sb[:], wk[:])
            nc.sync.dma_start(wq_sb[:], wq[:])
            # first x^T chunk + everything round 0 needs, then the rest
            nc.sync.dma_start(xt_sb[:, :, 0:512], xT[:, :, 0:512])
            nc.sync.dma_start(xqt_sb[:, :, 0:512], xQT[:, :, 0:512])
            nc.sync.dma_start(wv_sb[:], wv[:])
            nc.sync.dma_start(mt[0][:], msk[0])
            nc.sync.dma_start(sel_sb[:], sel[:])
            for nb in range(1, T // 512):
                nc.sync.dma_start(xt_sb[:, :, nb * 512:(nb + 1) * 512],
                                  xT[:, :, nb * 512:(nb + 1) * 512])
            if with_bias:
                bqk_sb = pp.tile([128, DC, 2], dt.float32, tag="bqk")
                bvo_sb = pp.tile([1, 2, D], dt.bfloat16, tag="bvo")
                ones1_sb = pp.tile([1, 128], dt.bfloat16, tag="ones1")
                nc.sync.dma_start(bqk_sb[:], bqkd[:])
                nc.sync.dma_start(bvo_sb[:], bvo[:])
                nc.sync.dma_start(ones1_sb[:], ones1[:])
            for i in range(1, NMB):        # jq0 masks + jq1 group 0
                nc.sync.dma_start(mt[i][:], msk[i])
            nc.sync.dma_start(xqt_sb[:, :, 512:1024], xQT[:, :, 512:1024])
            nc.sync.dma_start(wo_sb[:], wo[:])
            # ones column of V_aug (denominator accumulator)
            for g in range(4):
                nc.vector.memset(v_g[g][:, :, :, DH:DH + 1], 1.0)

            # ---- projection tile emitters (PSUM via psX, copies on DVE) ----
            def kq_tile(w_sb, src_sb, out_ap, nb, bi, fo):
                ps = psX.tile([128, 512], dt.float32, tag="psX")
                for dc in range(DC):
                    nc.tensor.matmul(
                        ps[:],
                        w_sb[:, dc, fo * 128:(fo + 1) * 128],
                        src_sb[:, dc, nb * 512:(nb + 1) * 512],
                        start=(dc == 0), stop=(dc == DC - 1),
                    )
                if with_bias:
                    nc.vector.tensor_scalar(
                        out_ap, ps[:], bqk_sb[:, fo, bi:bi + 1], 0.0,
                        mybir.AluOpType.add, mybir.AluOpType.bypass,
                    )
                else:
                    nc.vector.tensor_copy(out_ap, ps[:])

            def k_tile(fo, nb):
                kq_tile(wk_sb, xt_sb, kt_f[fo][:, nb * 512:(nb + 1) * 512], nb, 1, fo)

            def q_tile(fo, nb):
                kq_tile(wq_sb, xqt_sb, qt_f[fo][:, nb * 512:(nb + 1) * 512], nb, 0, fo)

            def v_tile(tt):
                ps = psX.tile([128, 512], dt.float32, tag="psX")
                for dc in range(DC):
                    nc.tensor.matmul(
                        ps[:],
                        xt_sb[:, dc, tt * 128:(tt + 1) * 128],
                        wv_sb[:, dc, :],
                        start=(dc == 0),
                        stop=(not with_bias and dc == DC - 1),
                    )
                if with_bias:
                    nc.tensor.matmul(ps[:], ones1_sb[:], bvo_sb[:, 0, :],
                                     start=False, stop=True)
                nc.vector.tensor_copy(
                    v_g[tt // 8][:, tt % 8, :, 0:DH],
                    ps[:].rearrange("p (h f) -> p h f", h=H))

            # just-in-time emission bookkeeping
            emitted = set()

            def need(item):
                if item in emitted:
                    return
                emitted.add(item)
                kind, a, b = item
                if kind == "k":
                    k_tile(a, b)
                elif kind == "q":
                    q_tile(a, b)
                else:
                    v_tile(a)

            # background queue: work not strictly required yet, drained at
            # a bounded rate during rounds so the PE never starves ScalarE
            # for long and later pairs' inputs are ready ahead of use.
            bg = []
            for fo in range(1, DC):
                bg.append(("q", fo, 0))
                for nb in range(T // 512):
                    bg.append(("k", fo, nb))
            for fo in range(DC):
                bg.append(("q", fo, 1))
            bgi = [0]

            def drain_bg(n):
                while n > 0 and bgi[0] < len(bg):
                    need(bg[bgi[0]])
                    bgi[0] += 1
                    n -= 1

            # ---- fused attention rounds ----
            for jq in range(2):
                qs = slice(jq * 512, (jq + 1) * 512)
                for pr in range(H // 2):
                    need(("q", pr, jq))
                    ot_e = psO.tile([DH + 1, 512], dt.float32, tag="ote")
                    ot_o = psO.tile([DH + 1, 512], dt.float32, tag="oto")
                    for kc in range(KC):
                        need(("k", pr, kc // 4))
                        need(("v", kc, 0))
                        if jq == 0:
                            if pr == 0:
                                # keep ~2 chunks of V lookahead, then keys
                                if kc + 2 < KC:
                                    need(("v", kc + 2, 0))
                                if kc % 4 == 1 and kc // 4 + 1 < 8:
                                    need(("k", 0, kc // 4 + 1))
                                if kc % 2 == 1:
                                    drain_bg(1)
                            else:
                                drain_bg(1)
                        ks = slice(kc * 128, (kc + 1) * 128)
                        sp = psS.tile([128, 1024], dt.float32, tag="sp")
                        nc.tensor.matmul(
                            sp[:, 0:512],
                            kt_f[pr][0:64, ks], qt_f[pr][0:64, qs],
                            start=True, stop=True)
                        nc.tensor.matmul(
                            sp[:, 512:1024],
                            kt_f[pr][64:128, ks], qt_f[pr][64:128, qs],
                            start=True, stop=True)
                        p_sb = pP.tile([128, 1024], dt.bfloat16, tag="p")
                        nc.scalar.activation(p_sb[:], sp[:], AF.Exp,
                                             scale=0.125)
                        mb = mt[(jq * 8 + kc // 4) % NMB]
                        pv = p_sb[:].rearrange("p (a b) -> p a b", a=2)
                        nc.vector.tensor_mul(
                            pv, pv,
                            mb[:, kc % 4, :][:, None, :]
                            .to_broadcast((128, 2, 512)))
                        nc.tensor.matmul(ot_e[:], v_g[kc // 8][:, kc % 8, 2 * pr, :],
                                         p_sb[:, 0:512],
                                         start=(kc == 0), stop=(kc == KC - 1))
                        nc.tensor.matmul(ot_o[:], v_g[kc // 8][:, kc % 8, 2 * pr + 1, :],
                                         p_sb[:, 512:1024],
                                         start=(kc == 0), stop=(kc == KC - 1))
                        if jq == 0 and pr == 3:
                            # refill mask buffer with jq1's next group once
                            # its last jq0 reader (this round) is done
                            g = kc // 4
                            if kc % 4 == 3 and g >= 1:
                                nc.sync.dma_start(mt[(8 + g) % NMB][:], msk[8 + g])

                    # normalize both heads by their denominator rows:
                    # DVE reciprocal straight from PSUM, then a ones-row
                    # matmul broadcasts the reciprocal across 64 partitions.
                    for i, ot_ps in ((0, ot_e), (1, ot_o)):
                        den = pN.tile([1, 512], dt.float32, tag="den")
                        nc.vector.tensor_copy(den[:], ot_ps[DH:DH + 1, :])
                        rec = pN.tile([1, 512], dt.float32, tag="rec")
                        nc.vector.reciprocal_approx_fast(rec[:], den[:])
                        rech = pN.tile([1, 512], dt.float16, tag="rech")
                        nc.vector.tensor_copy(rech[:], rec[:])
                        bc = psX.tile([64, 512], dt.float32, tag="psX")
                        nc.tensor.matmul(bc[:], sel_sb[0:1, 0:64], rech[:],
                                         start=True, stop=True)
                        bcs = pN.tile([64, 512], dt.float32, tag="bcs")
                        nc.vector.tensor_copy(bcs[:], bc[:])
                        nc.vector.tensor_mul(
                            ot_f[pr][i * 64:(i + 1) * 64, qs],
                            ot_ps[0:DH, :], bcs[:])

                # output projection for this query half (overlaps the next
                # half's attention rounds on the PE)
                for tt in range(jq * 4, (jq + 1) * 4):
                    ps = psX.tile([128, 512], dt.float32, tag="psX")
                    for dc in range(DC):
                        nc.tensor.matmul(
                            ps[:],
                            ot_f[dc][:, tt * 128:(tt + 1) * 128],
                            wo_sb[:, dc, :],
                            start=(dc == 0),
                            stop=(with_bias is False and dc == DC - 1))
                    if with_bias:
                        nc.tensor.matmul(ps[:], ones1_sb[:],
                                         bvo_sb[:, 1, :],
                                         start=False, stop=True)
                    os = pC.tile([128, 512], dt.float32, tag="os")
                    nc.vector.tensor_copy(os[:], ps[:])
                    nc.sync.dma_start(out[tt * 128:(tt + 1) * 128, :], os[:])

    nc.compile()
    return nc


def _get_nc(with_bias: bool):
    if with_bias not in _BUILT:
        _BUILT[with_bias] = _build(with_bias)
    return _BUILT[with_bias]


def _prep_inputs(x, Wq, bq, Wk, bk, Wv, bv, Wo, bo, mask, with_bias):
    bf16 = ml_dtypes.bfloat16

    shared = {}
    for name, W in (("wq", Wq), ("wk", Wk), ("wv", Wv), ("wo", Wo)):
        shared[name] = np.ascontiguousarray(
            np.asarray(W, np.float32).astype(bf16)
            .reshape(DC, 128, D).transpose(1, 0, 2))
    selm = np.zeros((2, 128), np.float32)
    selm[0, 0:64] = 1.0
    selm[1, 64:128] = 1.0
    shared["sel"] = selm.astype(np.float16)
    if with_bias:
        shared["bqk"] = np.ascontiguousarray(np.stack(
            [np.asarray(bq, np.float32).reshape(DC, 128).T,
             np.asarray(bk, np.float32).reshape(DC, 128).T], axis=-1))
        shared["bvo"] = np.ascontiguousarray(np.stack(
            [np.asarray(bv, np.float32), np.asarray(bo, np.float32)]
        ).astype(bf16).reshape(1, 2, D))
        shared["ones1"] = np.ones((1, 128), np.float32).astype(bf16)

    maskT = np.asarray(mask).reshape(T, T).T          # (k, q)
    m01T = maskT.astype(np.float32)

    in_maps = []
    for c in range(N_CORES):
        b, qlo = c // 4, (c % 4) * QPC
        xTb = np.asarray(x[b], np.float32).T.astype(bf16)     # (D, T)
        m = dict(shared)
        m["xT"] = np.ascontiguousarray(
            xTb.reshape(DC, 128, T).transpose(1, 0, 2))
        m["xQT"] = np.ascontiguousarray(
            xTb[:, qlo:qlo + QPC].reshape(DC, 128, QPC).transpose(1, 0, 2))
        # (k, q) block of this core's queries -> [jq, g, 128, 4, 512]
        m01 = m01T[:, qlo:qlo + QPC].reshape(8, 4, 128, 2, 512)
        m["msk"] = np.ascontiguousarray(
            m01.transpose(3, 0, 2, 1, 4).reshape(16, 128, 4, 512)).astype(bf16)
        in_maps.append(m)
    return in_maps


def _run(inputs, trace=False):
    from concourse.bass_utils import run_bass_kernel_spmd

    with_bias = any(
        float(np.abs(np.asarray(inputs[k], np.float32)).max()) != 0.0
        for k in ("bq", "bk", "bv", "bo"))
    nc = _get_nc(with_bias)
    in_maps = _prep_inputs(
        inputs["x"], inputs["Wq"], inputs["bq"], inputs["Wk"], inputs["bk"],
        inputs["Wv"], inputs["bv"], inputs["Wo"], inputs["bo"],
        inputs["mask"], with_bias)
    res = run_bass_kernel_spmd(nc, in_maps, list(range(N_CORES)), trace=trace)
    O = np.empty((B, T, D), np.float32)
    for c in range(N_CORES):
        b, qlo = c // 4, (c % 4) * QPC
        O[b, qlo:qlo + QPC, :] = res.results[c]["out"]
    return O, res


def kernel(**inputs) -> np.ndarray:
    out, _ = _run(inputs, trace=False)
    return out


# revision 16
# speedup vs baseline: 1.0293x; 1.0060x over previous
"""Multi-head attention (B=2, T=4096, D=512, H=8) on 8 TRN2 NeuronCores.

Sharding: core c handles batch c//4 and query rows (c%4)*1024..+1024.
Heads stay together on a core; K/V are recomputed per core (no comm).

v2 layout: single fused stream — the K/Q/V projection tiles are emitted
just-in-time inside the attention round loop so the ScalarE exp stream
(the critical engine, ~1 elem/lane/cycle over 33.5M elems/core) starts
a few microseconds in instead of after the whole projection phase.
ScalarE runs exp only; every PSUM->SBUF copy and the softmax
normalization run on VectorE (reciprocal_approx_fast reads the
denominator row straight from PSUM; one 2-row selector matmul
broadcasts both heads' reciprocals across partitions).

Per-round dataflow (jq query half, pr head pair, kc key chunk):
  S^T pair = kt[0:64].T@qt[0:64] | kt[64:128].T@qt[64:128]  (row-tiled,
  concurrent 64-row PE tiles) -> exp(0.125 x) on ScalarE -> mask01
  multiply on VectorE (bf16, 2x mode) -> O^T(+denominator row) +=
  V_aug.T @ P accumulated over all 32 kc in PSUM.
"""

import sys

sys.path.insert(0, "/opt/trn_rl_repo")

import numpy as np
import ml_dtypes

B, T, D, H = 2, 4096, 512, 8
DH = D // H          # 64
N_CORES = 8
QPC = 1024           # query rows per core
DC = D // 128        # 4 partition chunks of the model dim
KC = T // 128        # 32 key chunks
NMB = 9              # rotating mask buffers of [128, 4kc, 512q]

_BUILT = {}


def _build(with_bias: bool):
    from concourse import bacc
    import concourse.mybir as mybir
    import concourse.tile as tile

    dt = mybir.dt
    AF = mybir.ActivationFunctionType

    nc = bacc.Bacc("TRN2", target_bir_lowering=False, debug=False,
                   num_devices=N_CORES)

    xT = nc.dram_tensor("xT", [128, DC, T], dt.bfloat16, kind="ExternalInput").ap()
    xQT = nc.dram_tensor("xQT", [128, DC, QPC], dt.bfloat16, kind="ExternalInput").ap()
    wq = nc.dram_tensor("wq", [128, DC, D], dt.bfloat16, kind="ExternalInput").ap()
    wk = nc.dram_tensor("wk", [128, DC, D], dt.bfloat16, kind="ExternalInput").ap()
    wv = nc.dram_tensor("wv", [128, DC, D], dt.bfloat16, kind="ExternalInput").ap()
    wo = nc.dram_tensor("wo", [128, DC, D], dt.bfloat16, kind="ExternalInput").ap()
    # mask chunks: index (jq*8+g) -> [128, 4 kc, 512 q] as 0/1 bf16
    msk = nc.dram_tensor("msk", [16, 128, 4, 512], dt.bfloat16, kind="ExternalInput").ap()
    sel = nc.dram_tensor("sel", [2, 128], dt.float16, kind="ExternalInput").ap()
    if with_bias:
        bqkd = nc.dram_tensor("bqk", [128, DC, 2], dt.float32, kind="ExternalInput").ap()
        bvo = nc.dram_tensor("bvo", [1, 2, D], dt.bfloat16, kind="ExternalInput").ap()
        ones1 = nc.dram_tensor("ones1", [1, 128], dt.bfloat16, kind="ExternalInput").ap()
    out = nc.dram_tensor("out", [QPC, D], dt.float32, kind="ExternalOutput").ap()

    with tile.TileContext(nc) as tc:
        with (
            tc.tile_pool(name="persist", bufs=1) as pp,
            tc.tile_pool(name="psS", bufs=2, space="PSUM") as psS,
            tc.tile_pool(name="psO", bufs=1, space="PSUM") as psO,
            tc.tile_pool(name="psX", bufs=2, space="PSUM") as psX,
            tc.tile_pool(name="pP", bufs=3) as pP,
            tc.tile_pool(name="pN", bufs=1) as pN,
            tc.tile_pool(name="pC", bufs=2) as pC,
        ):
            wq_sb = pp.tile([128, DC, D], dt.bfloat16, tag="wq")
            wk_sb = pp.tile([128, DC, D], dt.bfloat16, tag="wk")
            wv_sb = pp.tile([128, DC, D], dt.bfloat16, tag="wv")
            wo_sb = pp.tile([128, DC, D], dt.bfloat16, tag="wo")
            sel_sb = pp.tile([2, 128], dt.float16, tag="sel")
            xt_sb = pp.tile([128, DC, T], dt.bfloat16, tag="xt")
            xqt_sb = pp.tile([128, DC, QPC], dt.bfloat16, tag="xqt")
            kt_f = [pp.tile([128, T], dt.bfloat16, tag=f"kt{fo}", name=f"kt{fo}")
                    for fo in range(DC)]
            v_g = [pp.tile([128, KC // 4, H, DH + 1], dt.bfloat16, tag=f"v{g}", name=f"v{g}")
                   for g in range(4)]
            qt_f = [pp.tile([128, QPC], dt.bfloat16, tag=f"qt{fo}", name=f"qt{fo}")
                    for fo in range(DC)]
            ot_f = [pp.tile([128, QPC], dt.bfloat16, tag=f"ot{fo}", name=f"ot{fo}")
                    for fo in range(DC)]
            mt = [pp.tile([128, 4, 512], dt.bfloat16, tag=f"m{i}", name=f"m{i}")
                  for i in range(NMB)]

            # ---- DMA issue order = arrival order ----
            # smallest possible prefix for round 0: fo0 weight slices +
            # first x^T chunk, then everything else
            nc.sync.dma_start(wk_sb[:, :, 0:128], wk[:, :, 0:128])
            nc.sync.dma_start(wq_sb[:, :, 0:128], wq[:, :, 0:128])
            nc.sync.dma_start(xt_sb[:, :, 0:512], xT[:, :, 0:512])
            nc.sync.dma_start(xqt_sb[:, :, 0:512], xQT[:, :, 0:512])
            nc.sync.dma_start(mt[0][:], msk[0])
            nc.sync.dma_start(wv_sb[:], wv[:])
            nc.sync.dma_start(sel_sb[:], sel[:])
            nc.sync.dma_start(wk_sb[:, :, 128:512], wk[:, :, 128:512])
            nc.sync.dma_start(wq_sb[:, :, 128:512], wq[:, :, 128:512])
            for nb in range(1, T // 512):
                nc.sync.dma_start(xt_sb[:, :, nb * 512:(nb + 1) * 512],
                                  xT[:, :, nb * 512:(nb + 1) * 512])
            if with_bias:
                bqk_sb = pp.tile([128, DC, 2], dt.float32, tag="bqk")
                bvo_sb = pp.tile([1, 2, D], dt.bfloat16, tag="bvo")
                ones1_sb = pp.tile([1, 128], dt.bfloat16, tag="ones1")
                nc.sync.dma_start(bqk_sb[:], bqkd[:])
                nc.sync.dma_start(bvo_sb[:], bvo[:])
                nc.sync.dma_start(ones1_sb[:], ones1[:])
            for i in range(1, NMB):        # jq0 masks + jq1 group 0
                nc.sync.dma_start(mt[i][:], msk[i])
            nc.sync.dma_start(xqt_sb[:, :, 512:1024], xQT[:, :, 512:1024])
            nc.sync.dma_start(wo_sb[:], wo[:])
            # ones column of V_aug (denominator accumulator)
            for g in range(4):
                nc.vector.memset(v_g[g][:, :, :, DH:DH + 1], 1.0)

            # ---- projection tile emitters (PSUM via psX, copies on DVE) ----
            def kq_tile(w_sb, src_sb, out_ap, nb, bi, fo):
                ps = psX.tile([128, 512], dt.float32, tag="psX")
                for dc in range(DC):
                    nc.tensor.matmul(
                        ps[:],
                        w_sb[:, dc, fo * 128:(fo + 1) * 128],
                        src_sb[:, dc, nb * 512:(nb + 1) * 512],
                        start=(dc == 0), stop=(dc == DC - 1),
                    )
                if with_bias:
                    nc.vector.tensor_scalar(
                        out_ap, ps[:], bqk_sb[:, fo, bi:bi + 1], 0.0,
                        mybir.AluOpType.add, mybir.AluOpType.bypass,
                    )
                else:
                    nc.vector.tensor_copy(out_ap, ps[:])

            def k_tile(fo, nb):
                kq_tile(wk_sb, xt_sb, kt_f[fo][:, nb * 512:(nb + 1) * 512], nb, 1, fo)

            def q_tile(fo, nb):
                kq_tile(wq_sb, xqt_sb, qt_f[fo][:, nb * 512:(nb + 1) * 512], nb, 0, fo)

            def v_tile(tt):
                ps = psX.tile([128, 512], dt.float32, tag="psX")
                for dc in range(DC):
                    nc.tensor.matmul(
                        ps[:],
                        xt_sb[:, dc, tt * 128:(tt + 1) * 128],
                        wv_sb[:, dc, :],
                        start=(dc == 0),
                        stop=(not with_bias and dc == DC - 1),
                    )
                if with_bias:
                    nc.tensor.matmul(ps[:], ones1_sb[:], bvo_sb[:, 0, :],
                                     start=False, stop=True)
                nc.vector.tensor_copy(
                    v_g[tt // 8][:, tt % 8, :, 0:DH],
                    ps[:].rearrange("p (h f) -> p h f", h=H))

            # just-in-time emission bookkeeping
            emitted = set()

            def need(item):
                if item in emitted:
                    return
                emitted.add(item)
                kind, a, b = item
                if kind == "k":
                    k_tile(a, b)
                elif kind == "q":
                    q_tile(a, b)
                else:
                    v_tile(a)

            # background queue: projection work not strictly required yet,
            # drained at a bounded rate during rounds so later pairs'
            # inputs are ready ahead of their first use.
            bg = [("q", 0, 1)]
            for fo in range(1, DC):
                bg.append(("q", fo, 0))
                for nb in range(T // 512):
                    bg.append(("k", fo, nb))
                bg.append(("q", fo, 1))
            bgi = [0]

            def drain_bg(n):
                while n > 0 and bgi[0] < len(bg):
                    need(bg[bgi[0]])
                    bgi[0] += 1
                    n -= 1

            def norm_head(ot_ps, pr, i, qs):
                # stage 1: reciprocal of the denominator row (all DVE)
                den = pN.tile([1, 512], dt.float32, tag="den")
                nc.vector.tensor_copy(den[:], ot_ps[DH:DH + 1, :])
                rec = pN.tile([1, 512], dt.float32, tag="rec")
                nc.vector.reciprocal_approx_fast(rec[:], den[:])
                rech = pN.tile([1, 512], dt.float16, tag="rech")
                nc.vector.tensor_copy(rech[:], rec[:])

                def stage2():
                    # broadcast across 64 partitions + apply
                    bc = psX.tile([64, 512], dt.float32, tag="psX")
                    nc.tensor.matmul(bc[:], sel_sb[0:1, 0:64], rech[:],
                                     start=True, stop=True)
                    bcs = pN.tile([64, 512], dt.float32, tag="bcs")
                    nc.vector.tensor_copy(bcs[:], bc[:])
                    nc.vector.tensor_mul(
                        ot_f[pr][i * 64:(i + 1) * 64, qs],
                        ot_ps[0:DH, :], bcs[:])
                return stage2

            def out_proj_tile(tt):
                ps = psX.tile([128, 512], dt.float32, tag="psX")
                for dc in range(DC):
                    nc.tensor.matmul(
                        ps[:],
                        ot_f[dc][:, tt * 128:(tt + 1) * 128],
                        wo_sb[:, dc, :],
                        start=(dc == 0),
                        stop=(with_bias is False and dc == DC - 1))
                if with_bias:
                    nc.tensor.matmul(ps[:], ones1_sb[:],
                                     bvo_sb[:, 1, :],
                                     start=False, stop=True)
                os = pC.tile([128, 512], dt.float32, tag="os")
                nc.vector.tensor_copy(os[:], ps[:])
                nc.sync.dma_start(out[tt * 128:(tt + 1) * 128, :], os[:])

            # ---- fused attention rounds: jq outer, pr mid, kc inner.
            # PV lags scores by 2 rounds; the previous sweep's
            # normalization and the finished half's output projection are
            # deferred into the first rounds of the following sweep so
            # they never head-of-line-block the PE queue.
            deferred = []     # list of closures, a few emitted per round
            for jq in range(2):
                qs = slice(jq * 512, (jq + 1) * 512)
                for pr in range(H // 2):
                    need(("q", pr, jq))
                    ot_e = psO.tile([DH + 1, 512], dt.float32, tag="ote")
                    ot_o = psO.tile([DH + 1, 512], dt.float32, tag="oto")
                    pvq = []
                    for kc in range(KC):
                        need(("k", pr, kc // 4))
                        if pr == 0 and jq == 0:
                            need(("v", kc, 0))
                        elif jq == 0 and kc % 2 == 0:
                            drain_bg(1)
                        ks = slice(kc * 128, (kc + 1) * 128)
                        sp = psS.tile([128, 1024], dt.float32, tag="sp")
                        nc.tensor.matmul(
                            sp[:, 0:512],
                            kt_f[pr][0:64, ks], qt_f[pr][0:64, qs],
                            start=True, stop=True)
                        nc.tensor.matmul(
                            sp[:, 512:1024],
                            kt_f[pr][64:128, ks], qt_f[pr][64:128, qs],
                            start=True, stop=True)
                        p_sb = pP.tile([128, 1024], dt.bfloat16, tag="p")
                        nc.scalar.activation(p_sb[:], sp[:], AF.Exp,
                                             scale=0.125)
                        mb = mt[(jq * 8 + kc // 4) % NMB]
                        pv = p_sb[:].rearrange("p (a b) -> p a b", a=2)
                        nc.vector.tensor_mul(
                            pv, pv,
                            mb[:, kc % 4, :][:, None, :]
                            .to_broadcast((128, 2, 512)))

                        def pv_mm(kc=kc, p_sb=p_sb):
                            nc.tensor.matmul(
                                ot_e[:], v_g[kc // 8][:, kc % 8, 2 * pr, :],
                                p_sb[:, 0:512],
                                start=(kc == 0), stop=(kc == KC - 1))
                            nc.tensor.matmul(
                                ot_o[:], v_g[kc // 8][:, kc % 8, 2 * pr + 1, :],
                                p_sb[:, 512:1024],
                                start=(kc == 0), stop=(kc == KC - 1))
                        pvq.append(pv_mm)
                        if deferred and kc < 8:
                            deferred.pop(0)()
                        if len(pvq) > 2:
                            pvq.pop(0)()
                        if jq == 0 and pr == 3 and kc % 4 == 3 and kc // 4 >= 1:
                            # refill mask buffer with jq1's next group once
                            # its last jq0 reader (this round) is done
                            nc.sync.dma_start(mt[(8 + kc // 4) % NMB][:],
                                              msk[8 + kc // 4])
                    while pvq:
                        pvq.pop(0)()
                    assert not deferred
                    # queue this sweep's normalization for the next sweep
                    s2e = norm_head(ot_e, pr, 0, qs)
                    s2o = norm_head(ot_o, pr, 1, qs)
                    deferred.append(s2e)
                    deferred.append(s2o)
                    if pr == H // 2 - 1:
                        for tt in range(jq * 4, (jq + 1) * 4):
                            deferred.append(lambda tt=tt: out_proj_tile(tt))
            while deferred:
                deferred.pop(0)()

    nc.compile()
    return nc


def _get_nc(with_bias: bool):
    if with_bias not in _BUILT:
        _BUILT[with_bias] = _build(with_bias)
    return _BUILT[with_bias]


def _prep_inputs(x, Wq, bq, Wk, bk, Wv, bv, Wo, bo, mask, with_bias):
    bf16 = ml_dtypes.bfloat16

    shared = {}
    for name, W in (("wq", Wq), ("wk", Wk), ("wv", Wv), ("wo", Wo)):
        shared[name] = np.ascontiguousarray(
            np.asarray(W, np.float32).astype(bf16)
            .reshape(DC, 128, D).transpose(1, 0, 2))
    selm = np.zeros((2, 128), np.float32)
    selm[0, 0:64] = 1.0
    selm[1, 64:128] = 1.0
    shared["sel"] = selm.astype(np.float16)
    if with_bias:
        shared["bqk"] = np.ascontiguousarray(np.stack(
            [np.asarray(bq, np.float32).reshape(DC, 128).T,
             np.asarray(bk, np.float32).reshape(DC, 128).T], axis=-1))
        shared["bvo"] = np.ascontiguousarray(np.stack(
            [np.asarray(bv, np.float32), np.asarray(bo, np.float32)]
        ).astype(bf16).reshape(1, 2, D))
        shared["ones1"] = np.ones((1, 128), np.float32).astype(bf16)

    maskT = np.asarray(mask).reshape(T, T).T          # (k, q)
    m01T = maskT.astype(np.float32)

    in_maps = []
    for c in range(N_CORES):
        b, qlo = c // 4, (c % 4) * QPC
        xTb = np.asarray(x[b], np.float32).T.astype(bf16)     # (D, T)
        m = dict(shared)
        m["xT"] = np.ascontiguousarray(
            xTb.reshape(DC, 128, T).transpose(1, 0, 2))
        m["xQT"] = np.ascontiguousarray(
            xTb[:, qlo:qlo + QPC].reshape(DC, 128, QPC).transpose(1, 0, 2))
        # (k, q) block of this core's queries -> [jq, g, 128, 4, 512]
        m01 = m01T[:, qlo:qlo + QPC].reshape(8, 4, 128, 2, 512)
        m["msk"] = np.ascontiguousarray(
            m01.transpose(3, 0, 2, 1, 4).reshape(16, 128, 4, 512)).astype(bf16)
        in_maps.append(m)
    return in_maps


def _run(inputs, trace=False):
    from concourse.bass_utils import run_bass_kernel_spmd

    with_bias = any(
        float(np.abs(np.asarray(inputs[k], np.float32)).max()) != 0.0
        for k in ("bq", "bk", "bv", "bo"))
    nc = _get_nc(with_bias)
    in_maps = _prep_inputs(
        inputs["x"], inputs["Wq"], inputs["bq"], inputs["Wk"], inputs["bk"],
        inputs["Wv"], inputs["bv"], inputs["Wo"], inputs["bo"],
        inputs["mask"], with_bias)
    res = run_bass_kernel_spmd(nc, in_maps, list(range(N_CORES)), trace=trace)
    O = np.empty((B, T, D), np.float32)
    for c in range(N_CORES):
        b, qlo = c // 4, (c % 4) * QPC
        O[b, qlo:qlo + QPC, :] = res.results[c]["out"]
    return O, res


def kernel(**inputs) -> np.ndarray:
    out, _ = _run(inputs, trace=False)
    return out
